# revision 12
# baseline (speedup 1.0000x reference)
"""Weighted-BCE + masked-MSE loss on 8 Trainium2 cores (pure data parallel).

Math (t in {0,1} exactly):
  class_sum = sum(bce * w)
            = -(w1 * sum(t*ln p) + w0 * (sum(ln(1-p)) - sum(t*ln(1-p))))
  masked sq = (1-t)*(ro-rt)^2 = ((1-t)*(ro-rt))^2
  cnt_zeros = sum(1-t)
Each core reduces its shard to 5 scalars; host combines and applies weights.

Engine mix per tile (all native-ISA ops; custom-DVE STT/TTR ops have a
1-wait sync budget that Tile overflows, so they are avoided):
  ACT : l1=Ln(p); l0=Ln(1-p)[+accum]; m1t=1-t[+accum=count]; sq=Square[+accum]
  DVE : tl1=t*l1; tl0=t*l0; X-reduce(tl1); X-reduce(tl0)
  Pool: dd=ro-rt; mdd=m1t*dd
"""

import os
import sys

for _p in ("/opt/trn_rl_repo", "/root/.axon_site/_ro/trn_rl_repo"):
    if os.path.isdir(_p) and _p not in sys.path:
        sys.path.insert(0, _p)

import numpy as np

import concourse.bacc as bacc
import concourse.bass as bass
import concourse.bass_isa as bass_isa
import concourse.mybir as mybir
from concourse import tile
from concourse.bass_utils import run_bass_kernel_spmd

N = 16777216
NCORES = 8
NSHARD = N // NCORES  # 2097152
P = 128
F = 1024
NT = NSHARD // (P * F)  # 16

_F32 = mybir.dt.float32

LAST_RESULTS = None  # test harness peeks at exec_time_ns / trace path


def _build_nc():
    AF = mybir.ActivationFunctionType
    OP = mybir.AluOpType
    AX = mybir.AxisListType

    nc = bacc.Bacc(
        "TRN2", target_bir_lowering=False, debug=False, num_devices=NCORES
    )
    p_d = nc.dram_tensor("p", [NT, P, F], _F32, kind="ExternalInput")
    t_d = nc.dram_tensor("t", [NT, P, F], _F32, kind="ExternalInput")
    ro_d = nc.dram_tensor("ro", [NT, P, F], _F32, kind="ExternalInput")
    rt_d = nc.dram_tensor("rt", [NT, P, F], _F32, kind="ExternalInput")
    out_d = nc.dram_tensor("out", [1, 5], _F32, kind="ExternalOutput")

    with tile.TileContext(nc) as tc:
        with (
            tc.tile_pool(name="io", bufs=3) as io,
            tc.tile_pool(name="work", bufs=2) as work,
            tc.tile_pool(name="stats", bufs=1) as stats,
        ):
            acc_tl1 = stats.tile([P, NT], _F32)  # sum t*ln(p) per tile col
            acc_tl0 = stats.tile([P, NT], _F32)  # sum t*ln(1-p)
            acc_l0 = stats.tile([P, NT], _F32)  # sum ln(1-p)
            acc_sq = stats.tile([P, NT], _F32)  # sum (1-t)(ro-rt)^2
            acc_cnt = stats.tile([P, NT], _F32)  # sum (1-t)

            for i in range(NT):
                tp = io.tile([P, F], _F32, tag="p")
                tt = io.tile([P, F], _F32, tag="t")
                tro = io.tile([P, F], _F32, tag="ro")
                trt = io.tile([P, F], _F32, tag="rt")
                nc.sync.dma_start(tp[:], p_d[i, :, :])
                nc.sync.dma_start(tt[:], t_d[i, :, :])
                nc.sync.dma_start(tro[:], ro_d[i, :, :])
                nc.sync.dma_start(trt[:], rt_d[i, :, :])

                # ACT: logs; ln(1-p) reduces for free via accum_out
                l1 = work.tile([P, F], _F32, tag="l1")
                nc.scalar.activation(l1[:], tp[:], AF.Ln)
                l0 = work.tile([P, F], _F32, tag="l0")
                nc.scalar.activation(
                    l0[:], tp[:], AF.Ln, bias=1.0, scale=-1.0,
                    accum_out=acc_l0[:, i : i + 1],
                )
                # ACT: m1t = 1-t, accum gives the zero-count directly
                m1t = work.tile([P, F], _F32, tag="m1t")
                nc.scalar.activation(
                    m1t[:], tt[:], AF.Copy, bias=1.0, scale=-1.0,
                    accum_out=acc_cnt[:, i : i + 1],
                )

                # DVE: products + free-dim reduces (all native ops)
                tl1 = work.tile([P, F], _F32, tag="tl1")
                nc.vector.tensor_mul(tl1[:], tt[:], l1[:])
                nc.vector.tensor_reduce(
                    acc_tl1[:, i : i + 1], tl1[:], AX.X, OP.add
                )
                tl0 = work.tile([P, F], _F32, tag="tl0")
                nc.vector.tensor_mul(tl0[:], tt[:], l0[:])
                nc.vector.tensor_reduce(
                    acc_tl0[:, i : i + 1], tl0[:], AX.X, OP.add
                )

                # Pool: dd = ro-rt; mdd = (1-t)*dd
                dd = work.tile([P, F], _F32, tag="dd")
                nc.gpsimd.tensor_sub(dd[:], tro[:], trt[:])
                mdd = work.tile([P, F], _F32, tag="mdd")
                nc.gpsimd.tensor_mul(mdd[:], m1t[:], dd[:])

                # ACT: sq = mdd^2 with accum = masked MSE partial
                sq = work.tile([P, F], _F32, tag="sq")
                nc.scalar.activation(
                    sq[:], mdd[:], AF.Square, accum_out=acc_sq[:, i : i + 1]
                )

            # Fold per-tile partials into out[1,5]
            red5 = stats.tile([P, 8], _F32)
            for j, acc in enumerate((acc_tl1, acc_tl0, acc_l0, acc_sq, acc_cnt)):
                nc.vector.tensor_reduce(red5[:, j : j + 1], acc[:], AX.X, OP.add)
            tot5 = stats.tile([P, 8], _F32)
            nc.gpsimd.partition_all_reduce(
                tot5[:, 0:5], red5[:, 0:5], 128, bass_isa.ReduceOp.add
            )
            nc.sync.dma_start(out_d[:], tot5[0:1, 0:5])

    # Bacc pipeline: splits multi-wait sync (TRN2 allows 1 wait/inst),
    # lowers extended-ISA .instr bytes, register allocation, etc.
    nc.compile()
    return nc


def kernel(class_output, reg_output, class_target, reg_target, class_weights):
    global LAST_RESULTS
    nc = _build_nc()

    def shards(a):
        a = np.ascontiguousarray(np.asarray(a, dtype=np.float32))
        return [
            a[c * NSHARD : (c + 1) * NSHARD].reshape(NT, P, F) for c in range(NCORES)
        ]

    ps = shards(class_output)
    ts = shards(class_target)
    ros = shards(reg_output)
    rts = shards(reg_target)
    in_maps = [
        {"p": ps[c], "t": ts[c], "ro": ros[c], "rt": rts[c]} for c in range(NCORES)
    ]

    res = run_bass_kernel_spmd(nc, in_maps, core_ids=list(range(NCORES)))
    LAST_RESULTS = res

    parts = np.stack([np.asarray(res.results[c]["out"][0]) for c in range(NCORES)])
    tot = parts.sum(axis=0, dtype=np.float64)
    s_tl1, s_tl0, s_l0, s_sq, s_cnt = tot

    w0 = float(np.asarray(class_weights)[0, 0])
    w1 = float(np.asarray(class_weights)[0, 1])
    class_loss = -(w1 * s_tl1 + w0 * (s_l0 - s_tl0)) / N
    reg_loss = (s_sq / s_cnt) if s_cnt > 0 else 0.0
    return np.float32(0.5 * class_loss + 0.5 * reg_loss)
